# revision 10
# baseline (speedup 1.0000x reference)
"""Constrained Viterbi decoder on 8 Trainium2 NeuronCores.

Problem: B=16, T=1024, N=45. Output [B,T] int32 argmax-path tags.

Strategy (per core, pure batch data-parallel, 2 batch elements/core):
  - Host folds start/transition/end constraints into the potentials and
    zero-pads past each sequence length (zero matrices are max-plus-neutral
    for the decode, unlike the reference's eye-padding, and keep everything
    before `length` bit-exact).
  - Device runs, per batch element, a forward max-plus chain over t=0..512
    and a backward chain over t=1023..513 (meet in the middle halves the
    serial-latency-bound wall clock). Each chain alternates two step forms:
      even step:  tensor_scalar add (state column, per-partition scalar)
                  + gpsimd partition_all_reduce(max) -> state as broadcast
                  rows (the lane crossing)
      odd step:   tensor_tensor_reduce(add, max) with the broadcast-row
                  state -> state column (fused add+reduce, one DVE op)
    Even-t matrices are consumed in natural [i,j] orientation, odd-t
    matrices transposed [j,i]; the host prepares both so each matrix is
    read exactly once (memory-optimal).
  - Device streams out every alpha/beta vector; the host reconstructs the
    argmax path (backtrack via alphas on the left half, forward-track via
    betas on the right half). Max-plus is order-exact and each step does a
    single float add, so device alphas match the jax reference bit-for-bit
    and the decoded path is exact (validated: 0/16384 mismatches).
"""
import numpy as np

B, T, N = 16, 1024, 45
NCORES, BPC = 8, 2
HK = T // 2            # matrices per parity (512)
SFWD = HK // 2 + 1     # fwd pair-steps: 257 (t=0..512)
SBWD = HK // 2         # bwd pair-steps: 256 (t=1023..513)
RING = 32              # alpha-row history ring slots
CH = 32                # matrices per DMA chunk
NEG = -1.0e30
NINF = -1e5
PADDING_INDEX = -1

_CACHE = {}


def _register_viterbi_max():
    """Register a custom DVE op: out = in0 + in1, accum_out = max over free,
    seeded with -FLT_MAX. One DVE instruction per Viterbi step (the native
    TENSOR_TENSOR_REDUCE opcode faults on this runtime)."""
    from concourse import dve_ops
    from concourse.dve_spec import Spec, Src0, Src1, MaxNeg, maxx, lower, _has_src1
    from concourse.dve_uop import DveOpSpec

    name = "VITERBI_MAX"
    if name in dve_ops._SUB_OPCODE_FOR_NAME:
        return next(op for op in dve_ops.OPS if op.name == name)

    def _ref(in0, in1, c0, c1, c2):
        b = (in0.astype(np.float32) + in1).astype(np.float32)
        return b, b.reshape(b.shape[0], -1).max(axis=-1, keepdims=True)

    op = dve_ops.DveOp(
        name,
        Spec(body=Src0 + Src1, accum=maxx, accum_init=MaxNeg, reference=_ref),
        subdim=False,
        uops_sha={},
    )
    row = max(dve_ops._SUB_OPCODE_FOR_NAME.values()) + 1
    dve_ops.OPS.append(op)
    dve_ops.CUSTOM_DVE_SPECS[name] = op.spec
    dve_ops._SUB_OPCODE_FOR_NAME[name] = row
    for ver in ("v3", "v4"):
        spec_c = DveOpSpec(name=name, opcode=row, uops=lower(op.spec, ver=ver),
                           rd1_en=_has_src1(op.spec))
        op.uops_sha[ver] = spec_c.sha(ver)
    return op


def _build_bass():
    import concourse.mybir as mybir
    import concourse.bass_isa as bass_isa
    from concourse import bacc
    from concourse.tile import TileContext

    f32 = mybir.dt.float32
    ADD = mybir.AluOpType.add
    VM = _register_viterbi_max()

    nc = bacc.Bacc(None)
    nat = nc.declare_dram_parameter("nat", [BPC, N, HK, N], f32, isOutput=False)
    trn = nc.declare_dram_parameter("trn", [BPC, N, HK, N], f32, isOutput=False)
    ef = nc.declare_dram_parameter("ef", [BPC, SFWD, N], f32, isOutput=True)
    of = nc.declare_dram_parameter("of", [BPC, N, SFWD - 1], f32, isOutput=True)
    eb = nc.declare_dram_parameter("eb", [BPC, SBWD, N], f32, isOutput=True)
    ob = nc.declare_dram_parameter("ob", [BPC, N, SBWD], f32, isOutput=True)

    with TileContext(nc) as tc:
        with tc.tile_pool(name="main", bufs=1) as pool:
            zero = pool.tile([N, 1], f32, name="zero")
            nc.vector.memset(zero[:], 0.0)

            class Chain:
                pass

            chains = []
            for d in ("f", "b"):
                for b in range(BPC):
                    c = Chain()
                    c.d, c.b = d, b
                    c.colhist = pool.tile([N, 256], f32, name=f"colh_{d}{b}")
                    nc.vector.memset(c.colhist[:], 0.0)
                    c.scr_r = pool.tile([N, N], f32, name=f"scr_r_{d}{b}")
                    c.scr_o = pool.tile([N, N], f32, name=f"scr_o_{d}{b}")
                    c.ring = None
                    c.prev_ring = None
                    c.natc = None
                    c.prev_natc = None
                    c.trnc = None
                    c.prev_trnc = None
                    chains.append(c)

            def load(c, which, lo, cnt):
                t = pool.tile([N, cnt * N], f32, name=f"{which}_{c.d}{c.b}",
                              tag=f"{which}_{c.d}{c.b}", bufs=2)
                src = (nat if which == "nat" else trn)[c.b, :, lo:lo + cnt, :]
                nc.sync.dma_start(out=t[:], in_=src)
                return t

            def rotate_ring(c):
                c.prev_ring = c.ring
                c.ring = pool.tile([N, RING * N], f32, name=f"ring_{c.d}{c.b}",
                                   tag=f"ring_{c.d}{c.b}", bufs=2)

            def fwd_pair(c, s):
                if s % CH == 0:
                    c.prev_natc = c.natc
                    c.natc = load(c, "nat", s, min(CH, SFWD - s))
                    if s < SFWD - 1:
                        c.prev_trnc = c.trnc
                        c.trnc = load(c, "trn", s, CH)
                if s % RING == 0:
                    rotate_ring(c)
                if s > 0:
                    k = s - 1
                    slot = k % RING
                    ring = c.prev_ring if s % RING == 0 else c.ring
                    trnc = (c.prev_trnc if (s % CH == 0 and s < SFWD - 1)
                            else c.trnc)
                    loc = k % CH
                    nc.vector._custom_dve(
                        VM, out=c.scr_o[:],
                        in0=trnc[:, loc * N:(loc + 1) * N],
                        in1=ring[:, slot * N:(slot + 1) * N],
                        accum_out=c.colhist[:, k:k + 1])
                sc = zero[:, 0:1] if s == 0 else c.colhist[:, s - 1:s]
                loc = s % CH
                nc.vector.tensor_scalar(
                    c.scr_r[:], c.natc[:, loc * N:(loc + 1) * N], sc, None, ADD)
                slot = s % RING
                nc.gpsimd.partition_all_reduce(
                    out_ap=c.ring[:, slot * N:(slot + 1) * N], in_ap=c.scr_r[:],
                    channels=N, reduce_op=bass_isa.ReduceOp.max)
                if slot == RING - 1 or s == SFWD - 1:
                    r0 = s - slot
                    nc.sync.dma_start(out=ef[c.b, r0:s + 1, :],
                                      in_=c.ring[0:1, 0:(slot + 1) * N])

            def bwd_pair(c, s):
                cch = s // CH
                lo = HK - CH * (cch + 1)          # chunk k-range [lo, lo+CH)
                if s % CH == 0:
                    c.natc = load(c, "nat", lo, CH)
                    c.trnc = load(c, "trn", lo, CH)
                if s % RING == 0:
                    rotate_ring(c)
                loc = (CH - 1) - (s % CH)          # k = HK-1-s within chunk
                sc = zero[:, 0:1] if s == 0 else c.colhist[:, s - 1:s]
                nc.vector.tensor_scalar(
                    c.scr_r[:], c.trnc[:, loc * N:(loc + 1) * N], sc, None, ADD)
                slot = s % RING
                nc.gpsimd.partition_all_reduce(
                    out_ap=c.ring[:, slot * N:(slot + 1) * N], in_ap=c.scr_r[:],
                    channels=N, reduce_op=bass_isa.ReduceOp.max)
                if slot == RING - 1:
                    r0 = s - slot
                    nc.sync.dma_start(out=eb[c.b, r0:s + 1, :],
                                      in_=c.ring[0:1, 0:RING * N])
                if s < SBWD - 1:
                    nc.vector._custom_dve(
                        VM, out=c.scr_o[:],
                        in0=c.natc[:, loc * N:(loc + 1) * N],
                        in1=c.ring[:, slot * N:(slot + 1) * N],
                        accum_out=c.colhist[:, s:s + 1])

            for s in range(SFWD):
                for c in chains:
                    if c.d == "f":
                        fwd_pair(c, s)
                    elif s < SBWD:
                        bwd_pair(c, s)

            for c in chains:
                dst = of if c.d == "f" else ob
                nc.sync.dma_start(out=dst[c.b, :, :], in_=c.colhist[:, 0:256])

    if not nc.is_finalized():
        nc.finalize()
    return nc


def _prep(lp, lengths, start_c, end_c, trans_c):
    """Fold constraints into the potentials; zero-pad past each length.

    Add order matches the reference (trans, then start at t=0 which has no
    trans, then end) so every entry is bit-identical to the reference's clp
    at positions < length.
    """
    Bm, Tm, Nm = lp.shape[0], lp.shape[1], lp.shape[2]
    start_add = np.where(start_c, 0.0, NINF).astype(np.float32)
    end_add = np.where(end_c, 0.0, NINF).astype(np.float32)
    trans_add = np.where(trans_c, 0.0, NINF).astype(np.float32)
    arr = lp.astype(np.float32).copy()
    arr[:, 1:] += trans_add[None, None]
    pad = np.arange(Tm)[None, :] >= lengths[:, None]
    arr[pad] = 0.0
    arr[:, 0] += start_add[None, :]
    arr[np.arange(Bm), lengths - 1] += end_add[None, :]
    return arr


def _decode(arr, A, Bt, lengths):
    """A: [B, 513, N] alphas t=0..512; Bt: [B, 1024, N] betas (valid t>=512)."""
    Bm, Tm = arr.shape[0], arr.shape[1]
    TM = Tm // 2
    tags = np.full((Bm, Tm), PADDING_INDEX, np.int64)
    cur = np.argmax(A[:, TM] + Bt[:, TM], axis=1)
    tags[:, TM] = cur
    nxt = cur.copy()
    bidx = np.arange(Bm)
    for t in range(TM - 1, -1, -1):
        nxt = np.argmax(A[:, t] + arr[bidx, t + 1, :, nxt], axis=1)
        tags[:, t] = nxt
    prv = cur.copy()
    for t in range(TM + 1, Tm):
        prv = np.argmax(arr[bidx, t, prv, :] + Bt[:, t], axis=1)
        tags[:, t] = prv
    mask = np.arange(Tm)[None, :] < lengths[:, None]
    return np.where(mask, tags, PADDING_INDEX).astype(np.int32)


def kernel(log_potentials, lengths, start_constraints, end_constraints,
           transition_constraints):
    from concourse.bass_utils import run_bass_kernel_spmd

    lp = np.asarray(log_potentials, np.float32)
    lengths = np.asarray(lengths, np.int32)
    arr = _prep(lp, lengths, np.asarray(start_constraints),
                np.asarray(end_constraints), np.asarray(transition_constraints))

    in_maps = []
    for c in range(NCORES):
        pair = arr[c * BPC:(c + 1) * BPC]
        nat = np.ascontiguousarray(pair[:, 0::2].transpose(0, 2, 1, 3))
        trn = np.ascontiguousarray(pair[:, 1::2].transpose(0, 3, 1, 2))
        in_maps.append({"nat": nat, "trn": trn})

    if "nc" not in _CACHE:
        _CACHE["nc"] = _build_bass()
    res = run_bass_kernel_spmd(_CACHE["nc"], in_maps, core_ids=list(range(NCORES)))

    A = np.zeros((B, HK + 1, N), np.float32)
    Bt = np.zeros((B, T, N), np.float32)
    for c in range(NCORES):
        r = res.results[c]
        for b in range(BPC):
            g = c * BPC + b
            A[g, 0::2] = r["ef"][b]
            A[g, 1::2] = r["of"][b].T
            # bwd: eb[s] = beta_{1022-2s}; ob[:, s] = beta_{1021-2s}
            Bt[g, T - 2::-2][:SBWD] = r["eb"][b]
            Bt[g, T - 3::-2][:SBWD] = r["ob"][b].T
    return _decode(arr, A, Bt, lengths)


# revision 13
# speedup vs baseline: 1.4125x; 1.4125x over previous
"""Constrained Viterbi decoder on 8 Trainium2 NeuronCores.

Problem: B=16, T=1024, N=45. Output [B,T] int32 argmax-path tags.

Strategy (per core, pure batch data-parallel, 2 batch elements/core):
  - Host folds start/transition/end constraints into the potentials and
    zero-pads past each sequence length (zero matrices are max-plus-neutral
    for the decode, unlike the reference's eye-padding, and keep everything
    before `length` bit-exact).
  - Device runs two chain groups: a forward max-plus chain over t=0..512 and
    a backward chain over t=1023..513 (meet in the middle halves the serial
    wall clock). Both batch elements travel together. Each pair of timesteps:
      TT step:  tensor_tensor add of the pair-tile [45,(2,45)] with the
                state column pair broadcast via a stride-0 AP, then one
                gpsimd partition_all_reduce(max) over 45 partitions x 90
                free -> state as broadcast rows (the lane crossing)
      VM step:  custom DVE op VITERBI_MAX (out=in0+in1, accum=max over
                free, seeded -FLT_MAX) per batch -> state column
    Even-t matrices are consumed in natural [i,j] orientation, odd-t
    transposed [j,i]; both live in the same two pair-layout arrays and each
    matrix is read exactly once (memory-optimal).
  - Device streams out every alpha/beta vector; the host reconstructs the
    argmax path (backtrack via alphas on the left half, forward-track via
    betas on the right half). Max-plus is order-exact and each step does a
    single float add, so device alphas match the jax reference bit-for-bit
    and the decoded path is exact (validated: 0/16384 mismatches).
"""
import numpy as np

B, T, N = 16, 1024, 45
NCORES, BPC = 8, 2
HK = T // 2            # matrices per parity (512)
SFWD = HK // 2 + 1     # fwd pair-steps: 257 (t=0..512)
SBWD = HK // 2         # bwd pair-steps: 256 (t=1023..513)
RING = 32              # row-history ring slots
CH = 16                # matrices per DMA chunk
NINF = -1e5
PADDING_INDEX = -1
W = BPC * N            # 90

_CACHE = {}


def _register_viterbi_max():
    """Register a custom DVE op: out = in0 + in1, accum_out = max over free,
    seeded with -FLT_MAX. One DVE instruction per Viterbi step (the native
    TENSOR_TENSOR_REDUCE opcode faults on this runtime)."""
    from concourse import dve_ops
    from concourse.dve_spec import Spec, Src0, Src1, MaxNeg, maxx, lower, _has_src1
    from concourse.dve_uop import DveOpSpec

    name = "VITERBI_MAX"
    if name in dve_ops._SUB_OPCODE_FOR_NAME:
        return next(op for op in dve_ops.OPS if op.name == name)

    def _ref(in0, in1, c0, c1, c2):
        b = (in0.astype(np.float32) + in1).astype(np.float32)
        return b, b.reshape(b.shape[0], -1).max(axis=-1, keepdims=True)

    op = dve_ops.DveOp(
        name,
        Spec(body=Src0 + Src1, accum=maxx, accum_init=MaxNeg, reference=_ref),
        subdim=False,
        uops_sha={},
    )
    row = max(dve_ops._SUB_OPCODE_FOR_NAME.values()) + 1
    dve_ops.OPS.append(op)
    dve_ops.CUSTOM_DVE_SPECS[name] = op.spec
    dve_ops._SUB_OPCODE_FOR_NAME[name] = row
    for ver in ("v3", "v4"):
        spec_c = DveOpSpec(name=name, opcode=row, uops=lower(op.spec, ver=ver),
                           rd1_en=_has_src1(op.spec))
        op.uops_sha[ver] = spec_c.sha(ver)
    return op


def _build_bass():
    import concourse.mybir as mybir
    import concourse.bass_isa as bass_isa
    from concourse import bacc
    from concourse.tile import TileContext

    f32 = mybir.dt.float32
    ADD = mybir.AluOpType.add
    VM = _register_viterbi_max()

    nc = bacc.Bacc(None)
    # pair-layout inputs: natp[i, k, b, j] = arr[b, 2k, i, j]
    #                     trnp[j, k, b, i] = arr[b, 2k+1, i, j]
    natp = nc.declare_dram_parameter("natp", [N, HK, BPC, N], f32, isOutput=False)
    trnp = nc.declare_dram_parameter("trnp", [N, HK, BPC, N], f32, isOutput=False)
    ef = nc.declare_dram_parameter("ef", [SFWD, W], f32, isOutput=True)
    of = nc.declare_dram_parameter("of", [N, 2 * SFWD - 2], f32, isOutput=True)
    eb = nc.declare_dram_parameter("eb", [SBWD, W], f32, isOutput=True)
    ob = nc.declare_dram_parameter("ob", [N, 2 * SBWD - 2], f32, isOutput=True)

    with TileContext(nc) as tc:
        with tc.tile_pool(name="main", bufs=1) as pool:

            class G:
                pass

            groups = []
            for d in ("f", "b"):
                g = G()
                g.d = d
                g.nsteps = SFWD if d == "f" else SBWD
                # colhist cols [2s, 2s+2) = state pair entering TT step s
                g.colhist = pool.tile([N, 2 * g.nsteps + 2], f32, name=f"colh_{d}")
                nc.vector.memset(g.colhist[:], 0.0)
                g.rr = pool.tile([N, W], f32, name=f"rr_{d}")
                g.scr = [pool.tile([N, N], f32, name=f"scr_{d}{b}") for b in range(BPC)]
                g.ring = None
                g.prev_ring = None
                g.ttc = None        # chunk stream feeding TT steps
                g.prev_ttc = None
                g.vmc = None        # chunk stream feeding VM steps
                g.prev_vmc = None
                groups.append(g)

            def load(g, which, src, lo, cnt):
                t = pool.tile([N, cnt, BPC, N], f32, name=f"{which}_{g.d}",
                              tag=f"{which}_{g.d}", bufs=2)
                nc.sync.dma_start(out=t[:], in_=src[:, lo:lo + cnt, :, :])
                return t

            def pair(g, s):
                fwd = g.d == "f"
                c = s // CH
                if s % CH == 0:
                    # TT stream: fwd natp ascending; bwd trnp descending
                    g.prev_ttc = g.ttc
                    if fwd:
                        g.ttc = load(g, "tt", natp, s, min(CH, SFWD - s))
                    else:
                        g.ttc = load(g, "tt", trnp, HK - (c + 1) * CH, CH)
                    # VM stream: fwd trnp ascending; bwd natp descending
                    g.prev_vmc = g.vmc
                    if fwd:
                        if s < SFWD - 1:
                            g.vmc = load(g, "vm", trnp, s, CH)
                    else:
                        lo = HK + 1 - (c + 1) * CH
                        g.vmc = load(g, "vm", natp, lo, min(CH, HK - lo))
                if s % RING == 0:
                    g.prev_ring = g.ring
                    g.ring = pool.tile([N, RING * W], f32, name=f"ring_{g.d}",
                                       tag=f"ring_{g.d}", bufs=2)
                # --- VM step (odd t): state cols <- max over rows of prev AR
                if s > 0:
                    slot = (s - 1) % RING
                    ring = g.prev_ring if s % RING == 0 else g.ring
                    if fwd:
                        vmc = g.prev_vmc if s % CH == 0 else g.vmc
                        loc = (s - 1) % CH
                    else:
                        # k = HK - s; chunk c holds [HK+1-(c+1)CH, ...):
                        # local = CH-1-(s%CH) for every chunk (incl. the
                        # clamped chunk 0, whose tile is one tile short).
                        vmc = g.vmc
                        loc = CH - 1 - (s % CH)
                    for b in range(BPC):
                        nc.vector._custom_dve(
                            VM, out=g.scr[b][:],
                            in0=vmc[:, loc, b, :],
                            in1=ring[:, slot * W + b * N: slot * W + (b + 1) * N],
                            accum_out=g.colhist[:, 2 * s + b:2 * s + b + 1])
                # --- TT step (even t): rr = pair-tile + state-pair bcast
                loc = s % CH if fwd else CH - 1 - (s % CH)
                colpair = g.colhist[:, 2 * s:2 * s + 2]
                nc.vector.tensor_tensor(
                    g.rr[:].rearrange("p (b j) -> p b j", b=BPC),
                    g.ttc[:, loc, :, :],
                    colpair[:, :, None].broadcast_to([N, BPC, N]), ADD)
                slot = s % RING
                nc.gpsimd.partition_all_reduce(
                    out_ap=g.ring[:, slot * W:(slot + 1) * W], in_ap=g.rr[:],
                    channels=N, reduce_op=bass_isa.ReduceOp.max)
                if slot == RING - 1 or s == g.nsteps - 1:
                    r0 = s - slot
                    dst = ef if fwd else eb
                    nc.sync.dma_start(out=dst[r0:s + 1, :],
                                      in_=g.ring[0:1, 0:(slot + 1) * W])

            for s in range(SFWD):
                for g in groups:
                    if g.d == "f" or s < SBWD:
                        pair(g, s)

            for g in groups:
                dst = of if g.d == "f" else ob
                nc.sync.dma_start(out=dst[:, :],
                                  in_=g.colhist[:, 2:2 * g.nsteps])

    if not nc.is_finalized():
        nc.finalize()
    return nc


def _prep(lp, lengths, start_c, end_c, trans_c):
    """Fold constraints into the potentials; zero-pad past each length.

    Add order matches the reference (trans, then start at t=0 which has no
    trans, then end) so every entry is bit-identical to the reference's clp
    at positions < length.
    """
    Bm, Tm, Nm = lp.shape[0], lp.shape[1], lp.shape[2]
    start_add = np.where(start_c, 0.0, NINF).astype(np.float32)
    end_add = np.where(end_c, 0.0, NINF).astype(np.float32)
    trans_add = np.where(trans_c, 0.0, NINF).astype(np.float32)
    arr = lp.astype(np.float32).copy()
    arr[:, 1:] += trans_add[None, None]
    pad = np.arange(Tm)[None, :] >= lengths[:, None]
    arr[pad] = 0.0
    arr[:, 0] += start_add[None, :]
    arr[np.arange(Bm), lengths - 1] += end_add[None, :]
    return arr


def _decode(arr, A, Bt, lengths):
    """A: [B, 513, N] alphas t=0..512; Bt: [B, 1024, N] betas (valid t>=512)."""
    Bm, Tm = arr.shape[0], arr.shape[1]
    TM = Tm // 2
    tags = np.full((Bm, Tm), PADDING_INDEX, np.int64)
    cur = np.argmax(A[:, TM] + Bt[:, TM], axis=1)
    tags[:, TM] = cur
    nxt = cur.copy()
    bidx = np.arange(Bm)
    for t in range(TM - 1, -1, -1):
        nxt = np.argmax(A[:, t] + arr[bidx, t + 1, :, nxt], axis=1)
        tags[:, t] = nxt
    prv = cur.copy()
    for t in range(TM + 1, Tm):
        prv = np.argmax(arr[bidx, t, prv, :] + Bt[:, t], axis=1)
        tags[:, t] = prv
    mask = np.arange(Tm)[None, :] < lengths[:, None]
    return np.where(mask, tags, PADDING_INDEX).astype(np.int32)


def kernel(log_potentials, lengths, start_constraints, end_constraints,
           transition_constraints):
    from concourse.bass_utils import run_bass_kernel_spmd

    lp = np.asarray(log_potentials, np.float32)
    lengths = np.asarray(lengths, np.int32)
    arr = _prep(lp, lengths, np.asarray(start_constraints),
                np.asarray(end_constraints), np.asarray(transition_constraints))

    in_maps = []
    for c in range(NCORES):
        pair_arr = arr[c * BPC:(c + 1) * BPC]
        natp = np.ascontiguousarray(pair_arr[:, 0::2].transpose(2, 1, 0, 3))
        trnp = np.ascontiguousarray(pair_arr[:, 1::2].transpose(3, 1, 0, 2))
        in_maps.append({"natp": natp, "trnp": trnp})

    if "nc" not in _CACHE:
        _CACHE["nc"] = _build_bass()
    res = run_bass_kernel_spmd(_CACHE["nc"], in_maps, core_ids=list(range(NCORES)))

    A = np.zeros((B, HK + 1, N), np.float32)
    Bt = np.zeros((B, T, N), np.float32)
    for c in range(NCORES):
        r = res.results[c]
        for b in range(BPC):
            g = c * BPC + b
            # fwd: ef[s] = alpha_{2s} pair-rows; of col 2s-2+b = alpha_{2s-1}
            A[g, 0::2] = r["ef"][:, b * N:(b + 1) * N]
            A[g, 1::2] = r["of"][:, b::2].T
            # bwd: eb[s] = beta_{1022-2s}; ob col 2s-2+b = beta_{1023-2s}
            Bt[g, T - 2::-2][:SBWD] = r["eb"][:, b * N:(b + 1) * N]
            Bt[g, T - 3::-2][:SBWD - 1] = r["ob"][:, b::2].T
    return _decode(arr, A, Bt, lengths)


# revision 14
# speedup vs baseline: 1.4148x; 1.0016x over previous
"""Constrained Viterbi decoder on 8 Trainium2 NeuronCores.

Problem: B=16, T=1024, N=45. Output [B,T] int32 argmax-path tags.

Strategy (per core, pure batch data-parallel, 2 batch elements/core):
  - Host folds start/transition/end constraints into the potentials and
    zero-pads past each sequence length (zero matrices are max-plus-neutral
    for the decode, unlike the reference's eye-padding, and keep everything
    before `length` bit-exact).
  - Device runs two chain groups: a forward max-plus chain over t=0..512 and
    a backward chain over t=1023..513 (meet in the middle halves the serial
    wall clock). Both batch elements travel together. Each pair of timesteps:
      TT step:  tensor_tensor add of the pair-tile [45,(2,45)] with the
                state column pair broadcast via a stride-0 AP, then one
                gpsimd partition_all_reduce(max) over 45 partitions x 90
                free -> state as broadcast rows (the lane crossing)
      VM step:  custom DVE op VITERBI_MAX (out=in0+in1, accum=max over
                free, seeded -FLT_MAX) per batch -> state column
    Even-t matrices are consumed in natural [i,j] orientation, odd-t
    transposed [j,i]; both live in the same two pair-layout arrays and each
    matrix is read exactly once (memory-optimal).
  - Device streams out every alpha/beta vector; the host reconstructs the
    argmax path (backtrack via alphas on the left half, forward-track via
    betas on the right half). Max-plus is order-exact and each step does a
    single float add, so device alphas match the jax reference bit-for-bit
    and the decoded path is exact (validated: 0/16384 mismatches).
"""
import numpy as np

B, T, N = 16, 1024, 45
NCORES, BPC = 8, 2
HK = T // 2            # matrices per parity (512)
SFWD = HK // 2 + 1     # fwd pair-steps: 257 (t=0..512)
SBWD = HK // 2         # bwd pair-steps: 256 (t=1023..513)
RING = 64              # row-history ring slots
CH = 16                # matrices per DMA chunk
NINF = -1e5
PADDING_INDEX = -1
W = BPC * N            # 90

_CACHE = {}


def _register_viterbi_max():
    """Register a custom DVE op: out = in0 + in1, accum_out = max over free,
    seeded with -FLT_MAX. One DVE instruction per Viterbi step (the native
    TENSOR_TENSOR_REDUCE opcode faults on this runtime)."""
    from concourse import dve_ops
    from concourse.dve_spec import Spec, Src0, Src1, MaxNeg, maxx, lower, _has_src1
    from concourse.dve_uop import DveOpSpec

    name = "VITERBI_MAX"
    if name in dve_ops._SUB_OPCODE_FOR_NAME:
        return next(op for op in dve_ops.OPS if op.name == name)

    def _ref(in0, in1, c0, c1, c2):
        b = (in0.astype(np.float32) + in1).astype(np.float32)
        return b, b.reshape(b.shape[0], -1).max(axis=-1, keepdims=True)

    op = dve_ops.DveOp(
        name,
        Spec(body=Src0 + Src1, accum=maxx, accum_init=MaxNeg, reference=_ref),
        subdim=False,
        uops_sha={},
    )
    row = max(dve_ops._SUB_OPCODE_FOR_NAME.values()) + 1
    dve_ops.OPS.append(op)
    dve_ops.CUSTOM_DVE_SPECS[name] = op.spec
    dve_ops._SUB_OPCODE_FOR_NAME[name] = row
    for ver in ("v3", "v4"):
        spec_c = DveOpSpec(name=name, opcode=row, uops=lower(op.spec, ver=ver),
                           rd1_en=_has_src1(op.spec))
        op.uops_sha[ver] = spec_c.sha(ver)
    return op


def _build_bass():
    import concourse.mybir as mybir
    import concourse.bass_isa as bass_isa
    from concourse import bacc
    from concourse.tile import TileContext

    f32 = mybir.dt.float32
    ADD = mybir.AluOpType.add
    VM = _register_viterbi_max()

    nc = bacc.Bacc(None)
    # pair-layout inputs: natp[i, k, b, j] = arr[b, 2k, i, j]
    #                     trnp[j, k, b, i] = arr[b, 2k+1, i, j]
    natp = nc.declare_dram_parameter("natp", [N, HK, BPC, N], f32, isOutput=False)
    trnp = nc.declare_dram_parameter("trnp", [N, HK, BPC, N], f32, isOutput=False)
    ef = nc.declare_dram_parameter("ef", [SFWD, W], f32, isOutput=True)
    of = nc.declare_dram_parameter("of", [N, 2 * SFWD - 2], f32, isOutput=True)
    eb = nc.declare_dram_parameter("eb", [SBWD, W], f32, isOutput=True)
    ob = nc.declare_dram_parameter("ob", [N, 2 * SBWD - 2], f32, isOutput=True)

    with TileContext(nc) as tc:
        with tc.tile_pool(name="main", bufs=1) as pool:

            class G:
                pass

            groups = []
            for d in ("f", "b"):
                g = G()
                g.d = d
                g.nsteps = SFWD if d == "f" else SBWD
                # colhist cols [2s, 2s+2) = state pair entering TT step s
                g.colhist = pool.tile([N, 2 * g.nsteps + 2], f32, name=f"colh_{d}")
                nc.vector.memset(g.colhist[:], 0.0)
                g.rr = pool.tile([N, W], f32, name=f"rr_{d}")
                g.scr = [pool.tile([N, N], f32, name=f"scr_{d}{b}") for b in range(BPC)]
                g.ring = None
                g.prev_ring = None
                g.ttc = None        # chunk stream feeding TT steps
                g.prev_ttc = None
                g.vmc = None        # chunk stream feeding VM steps
                g.prev_vmc = None
                groups.append(g)

            def load(g, which, src, lo, cnt):
                t = pool.tile([N, cnt, BPC, N], f32, name=f"{which}_{g.d}",
                              tag=f"{which}_{g.d}", bufs=2)
                nc.sync.dma_start(out=t[:], in_=src[:, lo:lo + cnt, :, :])
                return t

            def pair(g, s):
                fwd = g.d == "f"
                c = s // CH
                if s % CH == 0:
                    # TT stream: fwd natp ascending; bwd trnp descending
                    g.prev_ttc = g.ttc
                    if fwd:
                        g.ttc = load(g, "tt", natp, s, min(CH, SFWD - s))
                    else:
                        g.ttc = load(g, "tt", trnp, HK - (c + 1) * CH, CH)
                    # VM stream: fwd trnp ascending; bwd natp descending
                    g.prev_vmc = g.vmc
                    if fwd:
                        if s < SFWD - 1:
                            g.vmc = load(g, "vm", trnp, s, CH)
                    else:
                        lo = HK + 1 - (c + 1) * CH
                        g.vmc = load(g, "vm", natp, lo, min(CH, HK - lo))
                if s % RING == 0:
                    g.prev_ring = g.ring
                    g.ring = pool.tile([N, RING * W], f32, name=f"ring_{g.d}",
                                       tag=f"ring_{g.d}", bufs=2)
                # --- VM step (odd t): state cols <- max over rows of prev AR
                if s > 0:
                    slot = (s - 1) % RING
                    ring = g.prev_ring if s % RING == 0 else g.ring
                    if fwd:
                        vmc = g.prev_vmc if s % CH == 0 else g.vmc
                        loc = (s - 1) % CH
                    else:
                        # k = HK - s; chunk c holds [HK+1-(c+1)CH, ...):
                        # local = CH-1-(s%CH) for every chunk (incl. the
                        # clamped chunk 0, whose tile is one tile short).
                        vmc = g.vmc
                        loc = CH - 1 - (s % CH)
                    for b in range(BPC):
                        nc.vector._custom_dve(
                            VM, out=g.scr[b][:],
                            in0=vmc[:, loc, b, :],
                            in1=ring[:, slot * W + b * N: slot * W + (b + 1) * N],
                            accum_out=g.colhist[:, 2 * s + b:2 * s + b + 1])
                # --- TT step (even t): rr = pair-tile + state-pair bcast
                loc = s % CH if fwd else CH - 1 - (s % CH)
                colpair = g.colhist[:, 2 * s:2 * s + 2]
                nc.vector.tensor_tensor(
                    g.rr[:].rearrange("p (b j) -> p b j", b=BPC),
                    g.ttc[:, loc, :, :],
                    colpair[:, :, None].broadcast_to([N, BPC, N]), ADD)
                slot = s % RING
                nc.gpsimd.partition_all_reduce(
                    out_ap=g.ring[:, slot * W:(slot + 1) * W], in_ap=g.rr[:],
                    channels=N, reduce_op=bass_isa.ReduceOp.max)
                if slot == RING - 1 or s == g.nsteps - 1:
                    r0 = s - slot
                    dst = ef if fwd else eb
                    nc.sync.dma_start(out=dst[r0:s + 1, :],
                                      in_=g.ring[0:1, 0:(slot + 1) * W])

            for s in range(SFWD):
                for g in groups:
                    if g.d == "f" or s < SBWD:
                        pair(g, s)

            for g in groups:
                dst = of if g.d == "f" else ob
                nc.sync.dma_start(out=dst[:, :],
                                  in_=g.colhist[:, 2:2 * g.nsteps])

    if not nc.is_finalized():
        nc.finalize()
    return nc


def _prep(lp, lengths, start_c, end_c, trans_c):
    """Fold constraints into the potentials; zero-pad past each length.

    Add order matches the reference (trans, then start at t=0 which has no
    trans, then end) so every entry is bit-identical to the reference's clp
    at positions < length.
    """
    Bm, Tm, Nm = lp.shape[0], lp.shape[1], lp.shape[2]
    start_add = np.where(start_c, 0.0, NINF).astype(np.float32)
    end_add = np.where(end_c, 0.0, NINF).astype(np.float32)
    trans_add = np.where(trans_c, 0.0, NINF).astype(np.float32)
    arr = lp.astype(np.float32).copy()
    arr[:, 1:] += trans_add[None, None]
    pad = np.arange(Tm)[None, :] >= lengths[:, None]
    arr[pad] = 0.0
    arr[:, 0] += start_add[None, :]
    arr[np.arange(Bm), lengths - 1] += end_add[None, :]
    return arr


def _decode(arr, A, Bt, lengths):
    """A: [B, 513, N] alphas t=0..512; Bt: [B, 1024, N] betas (valid t>=512)."""
    Bm, Tm = arr.shape[0], arr.shape[1]
    TM = Tm // 2
    tags = np.full((Bm, Tm), PADDING_INDEX, np.int64)
    cur = np.argmax(A[:, TM] + Bt[:, TM], axis=1)
    tags[:, TM] = cur
    nxt = cur.copy()
    bidx = np.arange(Bm)
    for t in range(TM - 1, -1, -1):
        nxt = np.argmax(A[:, t] + arr[bidx, t + 1, :, nxt], axis=1)
        tags[:, t] = nxt
    prv = cur.copy()
    for t in range(TM + 1, Tm):
        prv = np.argmax(arr[bidx, t, prv, :] + Bt[:, t], axis=1)
        tags[:, t] = prv
    mask = np.arange(Tm)[None, :] < lengths[:, None]
    return np.where(mask, tags, PADDING_INDEX).astype(np.int32)


def kernel(log_potentials, lengths, start_constraints, end_constraints,
           transition_constraints):
    from concourse.bass_utils import run_bass_kernel_spmd

    lp = np.asarray(log_potentials, np.float32)
    lengths = np.asarray(lengths, np.int32)
    arr = _prep(lp, lengths, np.asarray(start_constraints),
                np.asarray(end_constraints), np.asarray(transition_constraints))

    in_maps = []
    for c in range(NCORES):
        pair_arr = arr[c * BPC:(c + 1) * BPC]
        natp = np.ascontiguousarray(pair_arr[:, 0::2].transpose(2, 1, 0, 3))
        trnp = np.ascontiguousarray(pair_arr[:, 1::2].transpose(3, 1, 0, 2))
        in_maps.append({"natp": natp, "trnp": trnp})

    if "nc" not in _CACHE:
        _CACHE["nc"] = _build_bass()
    res = run_bass_kernel_spmd(_CACHE["nc"], in_maps, core_ids=list(range(NCORES)))

    A = np.zeros((B, HK + 1, N), np.float32)
    Bt = np.zeros((B, T, N), np.float32)
    for c in range(NCORES):
        r = res.results[c]
        for b in range(BPC):
            g = c * BPC + b
            # fwd: ef[s] = alpha_{2s} pair-rows; of col 2s-2+b = alpha_{2s-1}
            A[g, 0::2] = r["ef"][:, b * N:(b + 1) * N]
            A[g, 1::2] = r["of"][:, b::2].T
            # bwd: eb[s] = beta_{1022-2s}; ob col 2s-2+b = beta_{1023-2s}
            Bt[g, T - 2::-2][:SBWD] = r["eb"][:, b * N:(b + 1) * N]
            Bt[g, T - 3::-2][:SBWD - 1] = r["ob"][:, b::2].T
    return _decode(arr, A, Bt, lengths)
